# revision 30
# baseline (speedup 1.0000x reference)
"""Multi-head attention (B=4, N=2048, E=768, H=12, D=64) on 8 TRN2 NeuronCores.

Sharding: data-parallel on batch (4 batches x 2 cores each), tensor-parallel on
heads (6 heads per core).  Each core computes its heads' full NxN attention.
Partial output projections from the two cores of a batch are summed on the host.

Math simplifications (all exact):
  - softmax is shift invariant -> drop the +1.0 score bias and max-subtraction
  - K bias adds a per-query constant to every score row -> softmax invariant
  - V bias folded into the output bias on the host: b_eff = b_out + w_out @ bv
  - q scaling (1/8) folded into Wq and bq on the host

Device schedule (per core): one flat software pipeline over 192 "blocks"
(12 calls = 3 head-pairs x 4 query chunks of 512; 16 key blocks per call).
Scores are computed transposed with BOTH heads of the pair packed into one
[128,1024] PSUM tile (head A scores in cols 0:512, head B in 512:1024), so the
scalar engine runs exactly ONE [128,1024] exp per block -- its 260ns/instr
overhead is amortized and the exp stream (192 x ~1.1us = 214us) stays just
under the PE's total work.  Score tiles are double-buffered (2 tags x 2
banks), the O^T accumulators are [65,512] (1 bank each: 64 d rows + a
ones-column row-sum), and the remaining 2 PSUM banks are DEDICATED to
projection fills, so fills never insert a DVE-copy WAR into the QK/exp
critical path.  PV runs one block behind its exp, so the PE never stalls
waiting for the scalar engine.  Softmax normalization (reciprocal row-sums,
broadcast, per-head multiply) runs on the DVE/Pool engines at call
boundaries, off both critical engines.
"""

import sys

if "/opt/trn_rl_repo" not in sys.path:
    sys.path.insert(0, "/opt/trn_rl_repo")

import numpy as np

B, N, E = 4, 2048, 768
H, D = 12, 64
HPC = 6                     # heads per core
FQK = HPC * D               # 384 q (or k) features per core
NCORES = 8
SCALE = D ** -0.5
PRECISION = "bf16"

_CACHE = {}


def _build_bass():
    """Build the SPMD Bass program (same program on all 8 cores)."""
    if "nc" in _CACHE:
        return _CACHE["nc"]

    from contextlib import ExitStack

    import concourse.bass as bass
    import concourse.tile as tile
    from concourse import bacc, mybir

    f32 = mybir.dt.float32
    fmm = mybir.dt.bfloat16
    Exp = mybir.ActivationFunctionType.Exp
    Add = mybir.AluOpType.add

    nc = bacc.Bacc(
        "TRN2",
        target_bir_lowering=False,
        debug=False,
        num_devices=NCORES,
    )

    xT = nc.dram_tensor("xT", (E, N), fmm, kind="ExternalInput").ap()        # x[b].T
    wqkT = nc.dram_tensor("wqkT", (E, 2 * FQK), fmm, kind="ExternalInput").ap()
    bq = nc.dram_tensor("bq", (FQK, 1), f32, kind="ExternalInput").ap()
    wvT = nc.dram_tensor("wvT", (E, FQK), fmm, kind="ExternalInput").ap()
    woT = nc.dram_tensor("woT", (FQK, E), fmm, kind="ExternalInput").ap()
    yp = nc.dram_tensor("yp", (N, E), fmm, kind="ExternalOutput").ap()

    P = 128
    IC = 512                # query chunk (one call accumulates O^T for 512 q)
    NB = N // P             # 16 key blocks per call
    NPAIRS = HPC // 2       # 3 head pairs
    NCALLS = NPAIRS * 4     # pair-major: call = 4*p + chunk
    VB = NCALLS * NB        # 192 blocks

    with ExitStack() as ctx:
        tc = ctx.enter_context(tile.TileContext(nc))

        # ---- persistent tiles --------------------------------------------
        wpool = ctx.enter_context(tc.tile_pool(name="w", bufs=1))
        wqk_t = [wpool.tile([P, 2 * FQK], fmm, tag=f"wqk{t}", name=f"wqk{t}")
                 for t in range(6)]
        bqt = wpool.tile([P, 3], f32, tag="bq", name="bqt")
        xe_t = [wpool.tile([P, N], fmm, tag=f"xe{t}", name=f"xe{t}")
                for t in range(6)]
        wv_t = [wpool.tile([P, FQK], fmm, tag=f"wv{t}", name=f"wv{t}")
                for t in range(6)]
        wo_t = [wpool.tile([P, E], fmm, tag=f"wo{t}", name=f"wo{t}")
                for t in range(3)]
        scratch = wpool.tile([1, 2], f32, tag="scr", name="scratch")

        qk_pool = ctx.enter_context(tc.tile_pool(name="qk", bufs=1))
        # f-blocks 0..2 = q features (heads 2fb, 2fb+1), 3..5 = k features
        qkT_t = [
            qk_pool.tile([P, N], fmm, tag=f"qk{fb}", name=f"qkT{fb}")
            for fb in range(6)
        ]
        v_pool = ctx.enter_context(tc.tile_pool(name="v", bufs=1))
        # V' per key-block: [128 keys, 6*65] = per head 64 V cols + a ones col
        v_t = [
            v_pool.tile([P, HPC * 65], fmm, tag=f"v{nb}", name=f"vv{nb}")
            for nb in range(NB)
        ]
        oT_pool = ctx.enter_context(tc.tile_pool(name="oT", bufs=1))
        # pair p partitions 0:64 = head 2p, 64:128 = head 2p+1 (normalized)
        oT_t = [
            oT_pool.tile([P, N], fmm, tag=f"oT{p}", name=f"oT{p}")
            for p in range(NPAIRS)
        ]

        # PSUM: s0,s1 = double-buffered packed score tiles (2 banks each),
        # oA,oB = O^T accumulators (1 bank each), f0,f1 = fill banks.
        psum = ctx.enter_context(tc.tile_pool(name="ps", bufs=1, space="PSUM"))

        pt_pool = ctx.enter_context(tc.tile_pool(name="pt", bufs=3))
        nrm_pool = ctx.enter_context(tc.tile_pool(name="nrm", bufs=2))
        y_pool = ctx.enter_context(tc.tile_pool(name="y", bufs=3))
        part_pool = ctx.enter_context(tc.tile_pool(name="part", bufs=8))

        # ---- ones columns for the row-sum trick (rest written by v-proj) --
        for nb in range(NB):
            v3 = v_t[nb].rearrange("p (h c) -> p h c", c=65)
            nc.gpsimd.memset(v3[:, :, 64:65], 1.0)

        # ---- input DMAs: split across the two hardware DGE queues --------
        # sync queue: weights; activation queue: x^T (in token halves so the
        # first projection chains can start before the full x has landed);
        # the gpsimd SWDGE ring carries a slice of x's first half.
        for t in range(6):
            nc.sync.dma_start(wqk_t[t][:], wqkT[t * P:(t + 1) * P, :])
        for fb in range(3):
            nc.sync.dma_start(bqt[:, fb:fb + 1], bq[fb * P:(fb + 1) * P, :])
        for t in range(6):
            nc.sync.dma_start(wv_t[t][:], wvT[t * P:(t + 1) * P, :])
        for t in range(3):
            nc.scalar.dma_start(xe_t[t][:, 0:1024], xT[t * P:(t + 1) * P, 0:1024])
        for t in range(3, 6):
            nc.gpsimd.dma_start(xe_t[t][:, 0:1024], xT[t * P:(t + 1) * P, 0:1024])
        for n0 in (1024, 1536):
            for t in range(3):
                nc.scalar.dma_start(xe_t[t][:, n0:n0 + IC],
                                    xT[t * P:(t + 1) * P, n0:n0 + IC])
            for t in range(3, 6):
                nc.gpsimd.dma_start(xe_t[t][:, n0:n0 + IC],
                                    xT[t * P:(t + 1) * P, n0:n0 + IC])
        for t in range(3):
            nc.sync.dma_start(wo_t[t][:], woT[t * P:(t + 1) * P, :])

        # ones row for the PE-broadcast of reciprocal row-sums
        ones1 = wpool.tile([1, P], fmm, tag="ones1", name="ones1")
        nc.gpsimd.memset(ones1[:], 1.0)

        # warm the Exp activation table while DMAs run
        nc.gpsimd.memset(scratch[:], 0.0)
        nc.scalar.activation(scratch[:, 0:1], scratch[:, 1:2], Exp)

        # ---- projection fill units (dedicated psum banks f0/f1) ----------
        # Each fill is split into parts of <=2 matmuls placed on CONSECUTIVE
        # blocks: the per-block PE slack over the scalar's 1114ns exp pace is
        # only ~260ns, so a whole 6-matmul chain on one block starves the
        # exp stream.  Parts of one fill share a psum bank (accumulation),
        # assigned by interval (with a one-block cooldown for the DVE
        # copy-retire) so concurrent fills never collide.

        def qk_parts(fb, c4, tag):
            """QKV projection for f-block fb, token chunk c4: 3 parts."""
            st = {}
            n0 = c4 * IC

            def mk(k):
                def u():
                    if k == 0:
                        st["ps"] = psum.tile([P, IC], f32, tag=tag,
                                             name="ps_qk")
                    ps = st["ps"]
                    for et in (2 * k, 2 * k + 1):
                        nc.tensor.matmul(
                            ps[:],
                            lhsT=wqk_t[et][:, fb * P:(fb + 1) * P],
                            rhs=xe_t[et][:, n0:n0 + IC],
                            start=(et == 0),
                            stop=(et == 5),
                        )
                    if k == 2:
                        dst = qkT_t[fb][:, n0:n0 + IC]
                        if fb < 3:
                            nc.vector.tensor_scalar_add(dst, ps[:],
                                                        bqt[:, fb:fb + 1])
                        else:
                            nc.vector.tensor_copy(dst, ps[:])
                return u
            return [mk(k) for k in range(3)]

        def v_parts(nb, pp, tag):
            """V projection for key block nb, head pair pp (one part)."""
            def u():
                ps = psum.tile([P, 2 * D], f32, tag=tag, name="ps_v")
                for et in range(6):
                    nc.tensor.matmul(
                        ps[:],
                        lhsT=xe_t[et][:, nb * P:(nb + 1) * P],
                        rhs=wv_t[et][:, pp * 2 * D:(pp + 1) * 2 * D],
                        start=(et == 0),
                        stop=(et == 5),
                    )
                v3 = v_t[nb].rearrange("p (h c) -> p h c", c=65)
                nc.vector.tensor_copy(
                    v3[:, 2 * pp:2 * pp + 2, 0:64],
                    ps.rearrange("p (h c) -> p h c", c=64),
                )
            return [u]

        yts = {}

        def op_parts(c4, nb2, half, tag):
            """Out-projection half-block for query chunk c4 (one part)."""
            n0 = c4 * IC + nb2 * P
            f0 = half * 384

            def u():
                ps = psum.tile([P, 384], f32, tag=tag, name="ps_op")
                for dt3 in range(3):
                    nc.tensor.matmul(
                        ps[:],
                        lhsT=oT_t[dt3][:, n0:n0 + P],
                        rhs=wo_t[dt3][:, f0:f0 + 384],
                        start=(dt3 == 0),
                        stop=(dt3 == 2),
                    )
                if (c4, nb2) not in yts:
                    yts[(c4, nb2)] = y_pool.tile([P, E], fmm, tag="y",
                                                 name="yt")
                yt = yts[(c4, nb2)]
                nc.vector.tensor_copy(yt[:, f0:f0 + 384], ps[:])
                eng = nc.scalar if (2 * nb2 + half) % 2 == 1 else nc.sync
                eng.dma_start(yp[n0:n0 + P, f0:f0 + 384], yt[:, f0:f0 + 384])
            return [u]

        parts = {}

        def p1_parts(nb2, half, tag):
            """Tail out-proj (chunk c3) part 1: pairs 0/1, banked to SBUF."""
            def u():
                n0 = 3 * IC + nb2 * P
                f0 = half * 384
                ps = psum.tile([P, 384], f32, tag=tag, name="ps_p1")
                for dt3 in range(2):
                    nc.tensor.matmul(
                        ps[:],
                        lhsT=oT_t[dt3][:, n0:n0 + P],
                        rhs=wo_t[dt3][:, f0:f0 + 384],
                        start=(dt3 == 0),
                        stop=(dt3 == 1),
                    )
                pt = part_pool.tile([P, 384], f32, tag="part", name="ypart")
                parts[(nb2, half)] = pt
                nc.vector.tensor_copy(pt[:], ps[:])
            return [u]

        def op_p2(nb2, half, tag):
            """Tail out-proj part 2: pair 2 matmul + add + ship."""
            def u():
                n0 = 3 * IC + nb2 * P
                f0 = half * 384
                if (3, nb2) not in yts:
                    yts[(3, nb2)] = y_pool.tile([P, E], fmm, tag="y",
                                                name="yt")
                yt = yts[(3, nb2)]
                ps = psum.tile([P, 384], f32, tag=tag, name="ps_p2")
                nc.tensor.matmul(
                    ps[:],
                    lhsT=oT_t[2][:, n0:n0 + P],
                    rhs=wo_t[2][:, f0:f0 + 384],
                    start=True,
                    stop=True,
                )
                nc.vector.tensor_add(yt[:, f0:f0 + 384], ps[:],
                                     parts[(nb2, half)][:])
                eng = nc.scalar if (2 * nb2 + half) % 2 == 1 else nc.sync
                eng.dma_start(yp[n0:n0 + P, f0:f0 + 384], yt[:, f0:f0 + 384])
            return u

        # ---- fill schedule ------------------------------------------------
        # (builder, part_costs, release, deadline): parts go on consecutive
        # blocks in [release, deadline]; cost is matmul columns (PE time).
        QK3, V1, OP1, P11 = [1024] * 3, [768], [1152], [768]
        todo = []
        for c4 in range(2, 4):                    # kT pair 0 chunks 2..3
            todo.append((lambda t, c4=c4: qk_parts(3, c4, t),
                         QK3, 0, 4 * c4 - 1))
        for j in range(6, NB):                    # V pair 0 (0-5 in pre)
            todo.append((lambda t, j=j: v_parts(j, 0, t),
                         V1, 0 if j < 8 else 6, max(0, j - 1)))
        for j in range(NB):                       # V pairs 1/2 (later calls)
            todo.append((lambda t, j=j: v_parts(j, 1, t),
                         V1, 8, 20 + 2 * j))
        for j in range(NB):
            todo.append((lambda t, j=j: v_parts(j, 2, t),
                         V1, 8, 84 + 2 * j))
        for c4 in range(1, 4):                    # qT pair 0
            todo.append((lambda t, c4=c4: qk_parts(0, c4, t),
                         QK3, 0 if c4 < 2 else 8, 16 * c4 - 2))
        for c4 in range(4):                       # pair 1 k then q
            todo.append((lambda t, c4=c4: qk_parts(4, c4, t),
                         QK3, 0 if c4 < 2 else 8, 62 + 4 * c4))
        for c4 in range(4):
            todo.append((lambda t, c4=c4: qk_parts(1, c4, t),
                         QK3, 0 if c4 < 2 else 8, 62 + 16 * c4))
        for c4 in range(4):                       # pair 2 k then q
            todo.append((lambda t, c4=c4: qk_parts(5, c4, t),
                         QK3, 0 if c4 < 2 else 8, 126 + 4 * c4))
        for c4 in range(4):
            todo.append((lambda t, c4=c4: qk_parts(2, c4, t),
                         QK3, 0 if c4 < 2 else 8, 126 + 16 * c4))
        # out-proj chunk c: needs the pair-2 chunk-c normalization (blocks
        # 16*(9+c)+2 and +3); spread across that call.
        for c4 in range(3):
            for i in range(8):
                nb2, half = i // 2, i % 2
                todo.append((lambda t, a=c4, b=nb2, h=half: op_parts(a, b, h, t),
                             OP1, 16 * (9 + c4) + 6, 16 * (9 + c4) + 7 + 2 * i))
        for i in range(8):                        # tail p1s (calls 8-9)
            nb2, half = i // 2, i % 2
            todo.append((lambda t, b=nb2, h=half: p1_parts(b, h, t),
                         P11, 132, 140 + 2 * i))

        # Interval-based placement: EDF; for each fill pick the window in
        # [release, deadline] minimizing peak block load, requiring a psum
        # bank free over [start, end+1] (cooldown covers the DVE copy).
        load = [0] * VB
        tag_busy = {"f0": [(-10, -1)], "f1": [(-10, -1)]}  # pre-phase holds
        # normalization broadcasts: f0 two blocks after each boundary, f1
        # three blocks after (split so neither waits on the DVE chain)
        for k in range(1, NCALLS):
            tag_busy["f0"].append((k * NB + 4, k * NB + 4))
            tag_busy["f1"].append((k * NB + 5, k * NB + 5))
            load[k * NB + 4] += 512
            load[k * NB + 5] += 512
        fills = {}

        def tag_free(tag, b0, b1, soft=False):
            # hard: one spare block around each hold (covers the DVE
            # copy-retire); soft: abutting holds allowed (the later unit
            # then briefly WAR-waits the earlier fill's DVE copy)
            g = 0 if soft else 1
            return all(s > b1 + g or e < b0 - g for s, e in tag_busy[tag])

        # place the window-hungry multi-part chains first, singles after
        todo.sort(key=lambda t: (len(t[1]) == 1, t[3]))
        for builder, costs, rel, dl in todo:
            n = len(costs)
            best = None
            for b in range(rel, min(dl - n + 1, VB - n) + 1):
                score = max(load[b + i] + costs[i] for i in range(n))
                tag = next((t for t in ("f0", "f1")
                            if tag_free(t, b, b + n - 1)), None)
                if tag is None:
                    tag = next((t for t in ("f0", "f1")
                                if tag_free(t, b, b + n - 1, soft=True)),
                               None)
                    score += 600
                if tag is None:
                    continue
                if best is None or score <= best[0]:
                    best = (score, b, tag)
            assert best is not None, f"no window for fill dl={dl}"
            _, b, tag = best
            for i, u in enumerate(builder(tag)):
                fills.setdefault(b + i, []).append(u)
                load[b + i] += costs[i]
            tag_busy[tag].append((b, b + n - 1))

        # ---- pre-phase: minimum to start block 0 --------------------------
        for u in qk_parts(3, 0, "f0"):     # kT pair 0, keys 0:512
            u()
        for u in qk_parts(0, 0, "f1"):     # qT pair 0, queries 0:512
            u()
        for u in v_parts(0, 0, "f0"):
            u()
        for u in v_parts(1, 0, "f1"):
            u()
        for u in v_parts(2, 0, "f0"):
            u()
        for u in v_parts(3, 0, "f1"):
            u()
        for u in qk_parts(3, 1, "f0"):     # kT pair 0, keys 512:1024
            u()
        for u in v_parts(4, 0, "f1"):
            u()
        for u in v_parts(5, 0, "f1"):
            u()

        # ---- the flat pipeline --------------------------------------------
        def boundary(p, c4, oA, oB):
            """End of call (pair p, chunk c4): copy (unnormalized) O^T out
            of PSUM (both halves FIRST, so the next call's first PVs are
            not held up), then build bf16 reciprocal row-sum rows on the
            DVE.  Returns two deferred closures (run 2 and 3 blocks later,
            when the DVE chain has drained) that broadcast the reciprocals
            across partitions via a tiny PE ones-matmul into a fill bank
            and normalize oT in place."""
            q0 = c4 * IC
            last = (p == NPAIRS - 1 and c4 == 3)
            dsts, rcps = [], []
            if not last:
                for half, o_ps in ((0, oA), (1, oB)):
                    dst = oT_t[p][half * 64:(half + 1) * 64, q0:q0 + IC]
                    nc.vector.tensor_copy(dst, o_ps[0:64, :])
                    dsts.append(dst)
            for half, o_ps in ((0, oA), (1, oB)):
                rs = nrm_pool.tile([1, IC], f32, tag=f"rs{half}", name="rs")
                nc.vector.tensor_copy(rs[:], o_ps[64:65, :])
                rcp = nrm_pool.tile([1, IC], f32, tag=f"rcp{half}",
                                    name="rcp")
                nc.vector.reciprocal_approx_fast(rcp[:], rs[:])
                rcp16 = nrm_pool.tile([1, IC], fmm, tag=f"rcp16{half}",
                                      name="rcp16")
                nc.vector.tensor_copy(rcp16[:], rcp[:])
                rcps.append(rcp16)
            if last:
                for half, o_ps in ((0, oA), (1, oB)):
                    dst = oT_t[p][half * 64:(half + 1) * 64, q0:q0 + IC]
                    nc.vector.tensor_copy(dst, o_ps[0:64, :])
                    dsts.append(dst)

            def norm(half, tag):
                def n():
                    rb = psum.tile([P, IC], f32, tag=tag, name="rb")
                    nc.tensor.matmul(rb[:], lhsT=ones1[:], rhs=rcps[half][:],
                                     start=True, stop=True)
                    nc.vector.tensor_mul(
                        dsts[half], dsts[half],
                        rb[half * 64:(half + 1) * 64, :])
                return n
            return [norm(0, "f0"), norm(1, "f1")]

        cur = {}             # live per-call psum accumulators
        pending = []         # deferred normalization closures

        for vb in range(VB + 1):
            if vb < VB:
                c, jb = divmod(vb, NB)
                p, c4 = divmod(c, 4)
                q0 = c4 * IC
                j0 = jb * P
                qT = qkT_t[p]
                kT = qkT_t[3 + p]
                s = psum.tile([P, 2 * IC], f32, tag=f"s{vb % 2}", name="s")
                nc.tensor.matmul(
                    s[:, 0:IC],
                    lhsT=kT[0:64, j0:j0 + P],
                    rhs=qT[0:64, q0:q0 + IC],
                    start=True, stop=True,
                )
                nc.tensor.matmul(
                    s[:, IC:2 * IC],
                    lhsT=kT[64:128, j0:j0 + P],
                    rhs=qT[64:128, q0:q0 + IC],
                    start=True, stop=True,
                )
                pt = pt_pool.tile([P, 2 * IC], fmm, tag="pt", name="pt")
                nc.scalar.activation(pt[:], s[:], Exp)
                cur[vb] = (pt, p, jb)

            if vb >= 1:
                pt, pp, pjb = cur.pop(vb - 1)
                if pjb == 0:
                    cur["oA"] = psum.tile([65, IC], f32, tag="oA", name="oA")
                    cur["oB"] = psum.tile([65, IC], f32, tag="oB", name="oB")
                oA, oB = cur["oA"], cur["oB"]
                nc.tensor.matmul(
                    oA[:],
                    lhsT=v_t[pjb][:, (2 * pp) * 65:(2 * pp) * 65 + 65],
                    rhs=pt[:, 0:IC],
                    start=(pjb == 0),
                    stop=(pjb == NB - 1),
                )
                nc.tensor.matmul(
                    oB[:],
                    lhsT=v_t[pjb][:, (2 * pp + 1) * 65:(2 * pp + 1) * 65 + 65],
                    rhs=pt[:, IC:2 * IC],
                    start=(pjb == 0),
                    stop=(pjb == NB - 1),
                )
                if pjb == NB - 1:
                    pc = (vb - 1) // NB
                    pending.extend(boundary(pc // 4, pc % 4, oA, oB))

            if vb > VB - 1:
                while pending:
                    pending.pop(0)()
            elif pending and vb % NB >= 4:
                pending.pop(0)()

            for u in fills.get(vb, ()):
                u()

        # ---- tail: out-proj part 2 for the last query chunk ---------------
        TAIL_TAGS = ["f0", "f1", "s0", "s1", "oA", "oB"]
        for i in range(8):
            op_p2(i // 2, i % 2, TAIL_TAGS[i % 6])()

    nc.compile()
    _CACHE["nc"] = nc
    return nc


def _shard_inputs(x_q, w_qkv, b_qkv, w_out):
    """Build the 8 per-core input maps (numpy, host side)."""
    import ml_dtypes

    mm_np = ml_dtypes.bfloat16

    def cmm(a):
        return np.ascontiguousarray(a.astype(mm_np))

    in_maps = []
    for c in range(NCORES):
        b = c // 2
        h0 = (c % 2) * HPC
        qs = slice(h0 * D, h0 * D + FQK)
        ks = slice(E + h0 * D, E + h0 * D + FQK)
        vs = slice(2 * E + h0 * D, 2 * E + h0 * D + FQK)
        wq = w_qkv[qs] * SCALE                       # (384, 768)
        wk = w_qkv[ks]
        wv = w_qkv[vs]
        in_maps.append({
            "xT": cmm(x_q[b].T),                                     # (768, 2048)
            "wqkT": cmm(np.concatenate([wq, wk], axis=0).T),         # (768, 768)
            "bq": np.ascontiguousarray(
                (b_qkv[qs] * SCALE).reshape(FQK, 1)),                # (384, 1)
            "wvT": cmm(wv.T),                                        # (768, 384)
            "woT": cmm(w_out[:, h0 * D:h0 * D + FQK].T),
        })
    return in_maps


def kernel(x_q, w_qkv, b_qkv, w_out, b_out, _trace=False, _tmpdir=None):
    x_q = np.asarray(x_q, dtype=np.float32)
    w_qkv = np.asarray(w_qkv, dtype=np.float32)
    b_qkv = np.asarray(b_qkv, dtype=np.float32)
    w_out = np.asarray(w_out, dtype=np.float32)
    b_out = np.asarray(b_out, dtype=np.float32)

    from concourse.bass_utils import run_bass_kernel_spmd

    nc = _build_bass()
    in_maps = _shard_inputs(x_q, w_qkv, b_qkv, w_out)
    res = run_bass_kernel_spmd(
        nc, in_maps, core_ids=list(range(NCORES)), trace=_trace, tmpdir=_tmpdir
    )
    _CACHE["last_result"] = res

    # host unshard: sum the two head-shards of each batch, add the folded bias
    bv = b_qkv[2 * E:]                       # v bias, folded through w_out
    b_eff = b_out + w_out @ bv               # (768,)
    y = np.empty((B, N, E), dtype=np.float32)
    for b in range(B):
        y[b] = (
            res.results[2 * b]["yp"].astype(np.float32)
            + res.results[2 * b + 1]["yp"].astype(np.float32)
            + b_eff
        )
    return y


# revision 31
# speedup vs baseline: 1.0009x; 1.0009x over previous
"""Multi-head attention (B=4, N=2048, E=768, H=12, D=64) on 8 TRN2 NeuronCores.

Sharding: data-parallel on batch (4 batches x 2 cores each), tensor-parallel on
heads (6 heads per core).  Each core computes its heads' full NxN attention.
Partial output projections from the two cores of a batch are summed on the host.

Math simplifications (all exact):
  - softmax is shift invariant -> drop the +1.0 score bias and max-subtraction
  - K bias adds a per-query constant to every score row -> softmax invariant
  - V bias folded into the output bias on the host: b_eff = b_out + w_out @ bv
  - q scaling (1/8) folded into Wq and bq on the host

Device schedule (per core): one flat software pipeline over 192 "blocks"
(12 calls = 3 head-pairs x 4 query chunks of 512; 16 key blocks per call).
Scores are computed transposed with BOTH heads of the pair packed into one
[128,1024] PSUM tile (head A scores in cols 0:512, head B in 512:1024), so the
scalar engine runs exactly ONE [128,1024] exp per block -- its 260ns/instr
overhead is amortized and the exp stream (192 x ~1.1us = 214us) stays just
under the PE's total work.  Score tiles are double-buffered (2 tags x 2
banks), the O^T accumulators are [65,512] (1 bank each: 64 d rows + a
ones-column row-sum), and the remaining 2 PSUM banks are DEDICATED to
projection fills, so fills never insert a DVE-copy WAR into the QK/exp
critical path.  PV runs one block behind its exp, so the PE never stalls
waiting for the scalar engine.  Softmax normalization (reciprocal row-sums,
broadcast, per-head multiply) runs on the DVE/Pool engines at call
boundaries, off both critical engines.
"""

import sys

if "/opt/trn_rl_repo" not in sys.path:
    sys.path.insert(0, "/opt/trn_rl_repo")

import numpy as np

B, N, E = 4, 2048, 768
H, D = 12, 64
HPC = 6                     # heads per core
FQK = HPC * D               # 384 q (or k) features per core
NCORES = 8
SCALE = D ** -0.5
PRECISION = "bf16"

_CACHE = {}


def _build_bass():
    """Build the SPMD Bass program (same program on all 8 cores)."""
    if "nc" in _CACHE:
        return _CACHE["nc"]

    from contextlib import ExitStack

    import concourse.bass as bass
    import concourse.tile as tile
    from concourse import bacc, mybir

    f32 = mybir.dt.float32
    fmm = mybir.dt.bfloat16
    Exp = mybir.ActivationFunctionType.Exp
    Add = mybir.AluOpType.add

    nc = bacc.Bacc(
        "TRN2",
        target_bir_lowering=False,
        debug=False,
        num_devices=NCORES,
    )

    xT = nc.dram_tensor("xT", (E, N), fmm, kind="ExternalInput").ap()        # x[b].T
    wqkT = nc.dram_tensor("wqkT", (E, 2 * FQK), fmm, kind="ExternalInput").ap()
    bq = nc.dram_tensor("bq", (FQK, 1), f32, kind="ExternalInput").ap()
    wvT = nc.dram_tensor("wvT", (E, FQK), fmm, kind="ExternalInput").ap()
    woT = nc.dram_tensor("woT", (FQK, E), fmm, kind="ExternalInput").ap()
    yp = nc.dram_tensor("yp", (N, E), fmm, kind="ExternalOutput").ap()

    P = 128
    IC = 512                # query chunk (one call accumulates O^T for 512 q)
    NB = N // P             # 16 key blocks per call
    NPAIRS = HPC // 2       # 3 head pairs
    NCALLS = NPAIRS * 4     # pair-major: call = 4*p + chunk
    VB = NCALLS * NB        # 192 blocks

    with ExitStack() as ctx:
        tc = ctx.enter_context(tile.TileContext(nc))

        # ---- persistent tiles --------------------------------------------
        wpool = ctx.enter_context(tc.tile_pool(name="w", bufs=1))
        wqk_t = [wpool.tile([P, 2 * FQK], fmm, tag=f"wqk{t}", name=f"wqk{t}")
                 for t in range(6)]
        bqt = wpool.tile([P, 3], f32, tag="bq", name="bqt")
        xe_t = [wpool.tile([P, N], fmm, tag=f"xe{t}", name=f"xe{t}")
                for t in range(6)]
        wv_t = [wpool.tile([P, FQK], fmm, tag=f"wv{t}", name=f"wv{t}")
                for t in range(6)]
        wo_t = [wpool.tile([P, E], fmm, tag=f"wo{t}", name=f"wo{t}")
                for t in range(3)]
        scratch = wpool.tile([1, 2], f32, tag="scr", name="scratch")

        qk_pool = ctx.enter_context(tc.tile_pool(name="qk", bufs=1))
        # f-blocks 0..2 = q features (heads 2fb, 2fb+1), 3..5 = k features
        qkT_t = [
            qk_pool.tile([P, N], fmm, tag=f"qk{fb}", name=f"qkT{fb}")
            for fb in range(6)
        ]
        v_pool = ctx.enter_context(tc.tile_pool(name="v", bufs=1))
        # V' per key-block: [128 keys, 6*65] = per head 64 V cols + a ones col
        v_t = [
            v_pool.tile([P, HPC * 65], fmm, tag=f"v{nb}", name=f"vv{nb}")
            for nb in range(NB)
        ]
        oT_pool = ctx.enter_context(tc.tile_pool(name="oT", bufs=1))
        # pair p partitions 0:64 = head 2p, 64:128 = head 2p+1 (normalized)
        oT_t = [
            oT_pool.tile([P, N], fmm, tag=f"oT{p}", name=f"oT{p}")
            for p in range(NPAIRS)
        ]

        # PSUM: s0,s1 = double-buffered packed score tiles (2 banks each),
        # oA,oB = O^T accumulators (1 bank each), f0,f1 = fill banks.
        psum = ctx.enter_context(tc.tile_pool(name="ps", bufs=1, space="PSUM"))

        pt_pool = ctx.enter_context(tc.tile_pool(name="pt", bufs=4))
        nrm_pool = ctx.enter_context(tc.tile_pool(name="nrm", bufs=2))
        y_pool = ctx.enter_context(tc.tile_pool(name="y", bufs=3))
        part_pool = ctx.enter_context(tc.tile_pool(name="part", bufs=8))

        # ---- ones columns for the row-sum trick (rest written by v-proj) --
        for nb in range(NB):
            v3 = v_t[nb].rearrange("p (h c) -> p h c", c=65)
            nc.gpsimd.memset(v3[:, :, 64:65], 1.0)

        # ---- input DMAs: split across the two hardware DGE queues --------
        # sync queue: weights; activation queue: x^T (in token halves so the
        # first projection chains can start before the full x has landed);
        # the gpsimd SWDGE ring carries a slice of x's first half.
        for t in range(6):
            nc.sync.dma_start(wqk_t[t][:], wqkT[t * P:(t + 1) * P, :])
        for fb in range(3):
            nc.sync.dma_start(bqt[:, fb:fb + 1], bq[fb * P:(fb + 1) * P, :])
        for t in range(6):
            nc.sync.dma_start(wv_t[t][:], wvT[t * P:(t + 1) * P, :])
        for t in range(3):
            nc.scalar.dma_start(xe_t[t][:, 0:1024], xT[t * P:(t + 1) * P, 0:1024])
        for t in range(3, 6):
            nc.gpsimd.dma_start(xe_t[t][:, 0:1024], xT[t * P:(t + 1) * P, 0:1024])
        for n0 in (1024, 1536):
            for t in range(3):
                nc.scalar.dma_start(xe_t[t][:, n0:n0 + IC],
                                    xT[t * P:(t + 1) * P, n0:n0 + IC])
            for t in range(3, 6):
                nc.gpsimd.dma_start(xe_t[t][:, n0:n0 + IC],
                                    xT[t * P:(t + 1) * P, n0:n0 + IC])
        for t in range(3):
            nc.sync.dma_start(wo_t[t][:], woT[t * P:(t + 1) * P, :])

        # ones row for the PE-broadcast of reciprocal row-sums
        ones1 = wpool.tile([1, P], fmm, tag="ones1", name="ones1")
        nc.gpsimd.memset(ones1[:], 1.0)

        # warm the Exp activation table while DMAs run
        nc.gpsimd.memset(scratch[:], 0.0)
        nc.scalar.activation(scratch[:, 0:1], scratch[:, 1:2], Exp)

        # ---- projection fill units (dedicated psum banks f0/f1) ----------
        # Each fill is split into parts of <=2 matmuls placed on CONSECUTIVE
        # blocks: the per-block PE slack over the scalar's 1114ns exp pace is
        # only ~260ns, so a whole 6-matmul chain on one block starves the
        # exp stream.  Parts of one fill share a psum bank (accumulation),
        # assigned by interval (with a one-block cooldown for the DVE
        # copy-retire) so concurrent fills never collide.

        def qk_parts(fb, c4, tag):
            """QKV projection for f-block fb, token chunk c4: 3 parts."""
            st = {}
            n0 = c4 * IC

            def mk(k):
                def u():
                    if k == 0:
                        st["ps"] = psum.tile([P, IC], f32, tag=tag,
                                             name="ps_qk")
                    ps = st["ps"]
                    for et in (2 * k, 2 * k + 1):
                        nc.tensor.matmul(
                            ps[:],
                            lhsT=wqk_t[et][:, fb * P:(fb + 1) * P],
                            rhs=xe_t[et][:, n0:n0 + IC],
                            start=(et == 0),
                            stop=(et == 5),
                        )
                    if k == 2:
                        dst = qkT_t[fb][:, n0:n0 + IC]
                        if fb < 3:
                            nc.vector.tensor_scalar_add(dst, ps[:],
                                                        bqt[:, fb:fb + 1])
                        else:
                            nc.vector.tensor_copy(dst, ps[:])
                return u
            return [mk(k) for k in range(3)]

        def v_parts(nb, pp, tag):
            """V projection for key block nb, head pair pp (one part)."""
            def u():
                ps = psum.tile([P, 2 * D], f32, tag=tag, name="ps_v")
                for et in range(6):
                    nc.tensor.matmul(
                        ps[:],
                        lhsT=xe_t[et][:, nb * P:(nb + 1) * P],
                        rhs=wv_t[et][:, pp * 2 * D:(pp + 1) * 2 * D],
                        start=(et == 0),
                        stop=(et == 5),
                    )
                v3 = v_t[nb].rearrange("p (h c) -> p h c", c=65)
                nc.vector.tensor_copy(
                    v3[:, 2 * pp:2 * pp + 2, 0:64],
                    ps.rearrange("p (h c) -> p h c", c=64),
                )
            return [u]

        yts = {}

        def op_parts(c4, nb2, half, tag):
            """Out-projection half-block for query chunk c4 (one part)."""
            n0 = c4 * IC + nb2 * P
            f0 = half * 384

            def u():
                ps = psum.tile([P, 384], f32, tag=tag, name="ps_op")
                for dt3 in range(3):
                    nc.tensor.matmul(
                        ps[:],
                        lhsT=oT_t[dt3][:, n0:n0 + P],
                        rhs=wo_t[dt3][:, f0:f0 + 384],
                        start=(dt3 == 0),
                        stop=(dt3 == 2),
                    )
                if (c4, nb2) not in yts:
                    yts[(c4, nb2)] = y_pool.tile([P, E], fmm, tag="y",
                                                 name="yt")
                yt = yts[(c4, nb2)]
                nc.vector.tensor_copy(yt[:, f0:f0 + 384], ps[:])
                eng = nc.scalar if (2 * nb2 + half) % 2 == 1 else nc.sync
                eng.dma_start(yp[n0:n0 + P, f0:f0 + 384], yt[:, f0:f0 + 384])
            return [u]

        parts = {}

        def p1_parts(nb2, half, tag):
            """Tail out-proj (chunk c3) part 1: pairs 0/1, banked to SBUF."""
            def u():
                n0 = 3 * IC + nb2 * P
                f0 = half * 384
                ps = psum.tile([P, 384], f32, tag=tag, name="ps_p1")
                for dt3 in range(2):
                    nc.tensor.matmul(
                        ps[:],
                        lhsT=oT_t[dt3][:, n0:n0 + P],
                        rhs=wo_t[dt3][:, f0:f0 + 384],
                        start=(dt3 == 0),
                        stop=(dt3 == 1),
                    )
                pt = part_pool.tile([P, 384], f32, tag="part", name="ypart")
                parts[(nb2, half)] = pt
                nc.vector.tensor_copy(pt[:], ps[:])
            return [u]

        def op_p2(nb2, half, tag):
            """Tail out-proj part 2: pair 2 matmul + add + ship."""
            def u():
                n0 = 3 * IC + nb2 * P
                f0 = half * 384
                if (3, nb2) not in yts:
                    yts[(3, nb2)] = y_pool.tile([P, E], fmm, tag="y",
                                                name="yt")
                yt = yts[(3, nb2)]
                ps = psum.tile([P, 384], f32, tag=tag, name="ps_p2")
                nc.tensor.matmul(
                    ps[:],
                    lhsT=oT_t[2][:, n0:n0 + P],
                    rhs=wo_t[2][:, f0:f0 + 384],
                    start=True,
                    stop=True,
                )
                nc.vector.tensor_add(yt[:, f0:f0 + 384], ps[:],
                                     parts[(nb2, half)][:])
                eng = nc.scalar if (2 * nb2 + half) % 2 == 1 else nc.sync
                eng.dma_start(yp[n0:n0 + P, f0:f0 + 384], yt[:, f0:f0 + 384])
            return u

        # ---- fill schedule ------------------------------------------------
        # (builder, part_costs, release, deadline): parts go on consecutive
        # blocks in [release, deadline]; cost is matmul columns (PE time).
        QK3, V1, OP1, P11 = [1024] * 3, [768], [1152], [768]
        todo = []
        for c4 in range(2, 4):                    # kT pair 0 chunks 2..3
            todo.append((lambda t, c4=c4: qk_parts(3, c4, t),
                         QK3, 0, 4 * c4 - 1))
        for j in range(6, NB):                    # V pair 0 (0-5 in pre)
            todo.append((lambda t, j=j: v_parts(j, 0, t),
                         V1, 0 if j < 8 else 6, max(0, j - 1)))
        for j in range(NB):                       # V pairs 1/2 (later calls)
            todo.append((lambda t, j=j: v_parts(j, 1, t),
                         V1, 8, 20 + 2 * j))
        for j in range(NB):
            todo.append((lambda t, j=j: v_parts(j, 2, t),
                         V1, 8, 84 + 2 * j))
        for c4 in range(1, 4):                    # qT pair 0
            todo.append((lambda t, c4=c4: qk_parts(0, c4, t),
                         QK3, 0 if c4 < 2 else 8, 16 * c4 - 2))
        for c4 in range(4):                       # pair 1 k then q
            todo.append((lambda t, c4=c4: qk_parts(4, c4, t),
                         QK3, 0 if c4 < 2 else 8, 62 + 4 * c4))
        for c4 in range(4):
            todo.append((lambda t, c4=c4: qk_parts(1, c4, t),
                         QK3, 0 if c4 < 2 else 8, 62 + 16 * c4))
        for c4 in range(4):                       # pair 2 k then q
            todo.append((lambda t, c4=c4: qk_parts(5, c4, t),
                         QK3, 0 if c4 < 2 else 8, 126 + 4 * c4))
        for c4 in range(4):
            todo.append((lambda t, c4=c4: qk_parts(2, c4, t),
                         QK3, 0 if c4 < 2 else 8, 126 + 16 * c4))
        # out-proj chunk c: needs the pair-2 chunk-c normalization (blocks
        # 16*(9+c)+2 and +3); spread across that call.
        for c4 in range(3):
            for i in range(8):
                nb2, half = i // 2, i % 2
                todo.append((lambda t, a=c4, b=nb2, h=half: op_parts(a, b, h, t),
                             OP1, 16 * (9 + c4) + 6, 16 * (9 + c4) + 7 + 2 * i))
        for i in range(8):                        # tail p1s (calls 8-9)
            nb2, half = i // 2, i % 2
            todo.append((lambda t, b=nb2, h=half: p1_parts(b, h, t),
                         P11, 134, 142 + 2 * i))

        # Interval-based placement: EDF; for each fill pick the window in
        # [release, deadline] minimizing peak block load, requiring a psum
        # bank free over [start, end+1] (cooldown covers the DVE copy).
        load = [0] * VB
        tag_busy = {"f0": [(-10, -1)], "f1": [(-10, -1)]}  # pre-phase holds
        # normalization broadcasts: f0 two blocks after each boundary, f1
        # three blocks after (split so neither waits on the DVE chain)
        for k in range(1, NCALLS):
            tag_busy["f0"].append((k * NB + 4, k * NB + 4))
            tag_busy["f1"].append((k * NB + 5, k * NB + 5))
            load[k * NB + 4] += 512
            load[k * NB + 5] += 512
        fills = {}

        def tag_free(tag, b0, b1, soft=False):
            # hard: one spare block around each hold (covers the DVE
            # copy-retire); soft: abutting holds allowed (the later unit
            # then briefly WAR-waits the earlier fill's DVE copy)
            g = 0 if soft else 1
            return all(s > b1 + g or e < b0 - g for s, e in tag_busy[tag])

        # place the window-hungry multi-part chains first, singles after
        todo.sort(key=lambda t: (len(t[1]) == 1, t[3]))
        for builder, costs, rel, dl in todo:
            n = len(costs)
            best = None
            for b in range(rel, min(dl - n + 1, VB - n) + 1):
                score = max(load[b + i] + costs[i] for i in range(n))
                tag = next((t for t in ("f0", "f1")
                            if tag_free(t, b, b + n - 1)), None)
                if tag is None:
                    tag = next((t for t in ("f0", "f1")
                                if tag_free(t, b, b + n - 1, soft=True)),
                               None)
                    score += 600
                if tag is None:
                    continue
                if best is None or score <= best[0]:
                    best = (score, b, tag)
            assert best is not None, f"no window for fill dl={dl}"
            _, b, tag = best
            for i, u in enumerate(builder(tag)):
                fills.setdefault(b + i, []).append(u)
                load[b + i] += costs[i]
            tag_busy[tag].append((b, b + n - 1))

        # ---- pre-phase: minimum to start block 0 --------------------------
        for u in qk_parts(3, 0, "f0"):     # kT pair 0, keys 0:512
            u()
        for u in qk_parts(0, 0, "f1"):     # qT pair 0, queries 0:512
            u()
        for u in v_parts(0, 0, "f0"):
            u()
        for u in v_parts(1, 0, "f1"):
            u()
        for u in v_parts(2, 0, "f0"):
            u()
        for u in v_parts(3, 0, "f1"):
            u()
        for u in qk_parts(3, 1, "f0"):     # kT pair 0, keys 512:1024
            u()
        for u in v_parts(4, 0, "f1"):
            u()
        for u in v_parts(5, 0, "f1"):
            u()

        # ---- the flat pipeline --------------------------------------------
        def boundary(p, c4, oA, oB):
            """End of call (pair p, chunk c4): copy (unnormalized) O^T out
            of PSUM (both halves FIRST, so the next call's first PVs are
            not held up), then build bf16 reciprocal row-sum rows on the
            DVE.  Returns two deferred closures (run 2 and 3 blocks later,
            when the DVE chain has drained) that broadcast the reciprocals
            across partitions via a tiny PE ones-matmul into a fill bank
            and normalize oT in place."""
            q0 = c4 * IC
            last = (p == NPAIRS - 1 and c4 == 3)
            dsts, rcps = [], []
            if not last:
                for half, o_ps in ((0, oA), (1, oB)):
                    dst = oT_t[p][half * 64:(half + 1) * 64, q0:q0 + IC]
                    nc.vector.tensor_copy(dst, o_ps[0:64, :])
                    dsts.append(dst)
            for half, o_ps in ((0, oA), (1, oB)):
                rs = nrm_pool.tile([1, IC], f32, tag=f"rs{half}", name="rs")
                nc.vector.tensor_copy(rs[:], o_ps[64:65, :])
                rcp = nrm_pool.tile([1, IC], f32, tag=f"rcp{half}",
                                    name="rcp")
                nc.vector.reciprocal_approx_fast(rcp[:], rs[:])
                rcp16 = nrm_pool.tile([1, IC], fmm, tag=f"rcp16{half}",
                                      name="rcp16")
                nc.vector.tensor_copy(rcp16[:], rcp[:])
                rcps.append(rcp16)
            if last:
                for half, o_ps in ((0, oA), (1, oB)):
                    dst = oT_t[p][half * 64:(half + 1) * 64, q0:q0 + IC]
                    nc.vector.tensor_copy(dst, o_ps[0:64, :])
                    dsts.append(dst)
                    rcps[half] = rcps[half]

            def norm(half, tag):
                def n():
                    rb = psum.tile([P, IC], f32, tag=tag, name="rb")
                    nc.tensor.matmul(rb[:], lhsT=ones1[:], rhs=rcps[half][:],
                                     start=True, stop=True)
                    nc.vector.tensor_mul(
                        dsts[half], dsts[half],
                        rb[half * 64:(half + 1) * 64, :])
                return n
            return [norm(0, "f0"), norm(1, "f1")]

        cur = {}             # live per-call psum accumulators
        pending = []         # deferred normalization closures

        for vb in range(VB + 2):
            if vb < VB:
                c, jb = divmod(vb, NB)
                p, c4 = divmod(c, 4)
                q0 = c4 * IC
                j0 = jb * P
                qT = qkT_t[p]
                kT = qkT_t[3 + p]
                s = psum.tile([P, 2 * IC], f32, tag=f"s{vb % 2}", name="s")
                nc.tensor.matmul(
                    s[:, 0:IC],
                    lhsT=kT[0:64, j0:j0 + P],
                    rhs=qT[0:64, q0:q0 + IC],
                    start=True, stop=True,
                )
                nc.tensor.matmul(
                    s[:, IC:2 * IC],
                    lhsT=kT[64:128, j0:j0 + P],
                    rhs=qT[64:128, q0:q0 + IC],
                    start=True, stop=True,
                )
                pt = pt_pool.tile([P, 2 * IC], fmm, tag="pt", name="pt")
                nc.scalar.activation(pt[:], s[:], Exp)
                cur[vb] = (pt, p, jb)

            if vb >= 2:
                pt, pp, pjb = cur.pop(vb - 2)
                if pjb == 0:
                    cur["oA"] = psum.tile([65, IC], f32, tag="oA", name="oA")
                    cur["oB"] = psum.tile([65, IC], f32, tag="oB", name="oB")
                oA, oB = cur["oA"], cur["oB"]
                nc.tensor.matmul(
                    oA[:],
                    lhsT=v_t[pjb][:, (2 * pp) * 65:(2 * pp) * 65 + 65],
                    rhs=pt[:, 0:IC],
                    start=(pjb == 0),
                    stop=(pjb == NB - 1),
                )
                nc.tensor.matmul(
                    oB[:],
                    lhsT=v_t[pjb][:, (2 * pp + 1) * 65:(2 * pp + 1) * 65 + 65],
                    rhs=pt[:, IC:2 * IC],
                    start=(pjb == 0),
                    stop=(pjb == NB - 1),
                )
                if pjb == NB - 1:
                    pc = (vb - 2) // NB
                    pending.extend(boundary(pc // 4, pc % 4, oA, oB))

            if vb > VB - 1:
                while pending:
                    pending.pop(0)()
            elif pending and vb % NB >= 4:
                pending.pop(0)()

            for u in fills.get(vb, ()):
                u()

        # ---- tail: out-proj part 2 for the last query chunk ---------------
        TAIL_TAGS = ["f0", "f1", "s0", "s1", "oA", "oB"]
        for i in range(8):
            op_p2(i // 2, i % 2, TAIL_TAGS[i % 6])()

    nc.compile()
    _CACHE["nc"] = nc
    return nc


def _shard_inputs(x_q, w_qkv, b_qkv, w_out):
    """Build the 8 per-core input maps (numpy, host side)."""
    import ml_dtypes

    mm_np = ml_dtypes.bfloat16

    def cmm(a):
        return np.ascontiguousarray(a.astype(mm_np))

    in_maps = []
    for c in range(NCORES):
        b = c // 2
        h0 = (c % 2) * HPC
        qs = slice(h0 * D, h0 * D + FQK)
        ks = slice(E + h0 * D, E + h0 * D + FQK)
        vs = slice(2 * E + h0 * D, 2 * E + h0 * D + FQK)
        wq = w_qkv[qs] * SCALE                       # (384, 768)
        wk = w_qkv[ks]
        wv = w_qkv[vs]
        in_maps.append({
            "xT": cmm(x_q[b].T),                                     # (768, 2048)
            "wqkT": cmm(np.concatenate([wq, wk], axis=0).T),         # (768, 768)
            "bq": np.ascontiguousarray(
                (b_qkv[qs] * SCALE).reshape(FQK, 1)),                # (384, 1)
            "wvT": cmm(wv.T),                                        # (768, 384)
            "woT": cmm(w_out[:, h0 * D:h0 * D + FQK].T),
        })
    return in_maps


def kernel(x_q, w_qkv, b_qkv, w_out, b_out, _trace=False, _tmpdir=None):
    x_q = np.asarray(x_q, dtype=np.float32)
    w_qkv = np.asarray(w_qkv, dtype=np.float32)
    b_qkv = np.asarray(b_qkv, dtype=np.float32)
    w_out = np.asarray(w_out, dtype=np.float32)
    b_out = np.asarray(b_out, dtype=np.float32)

    from concourse.bass_utils import run_bass_kernel_spmd

    nc = _build_bass()
    in_maps = _shard_inputs(x_q, w_qkv, b_qkv, w_out)
    res = run_bass_kernel_spmd(
        nc, in_maps, core_ids=list(range(NCORES)), trace=_trace, tmpdir=_tmpdir
    )
    _CACHE["last_result"] = res

    # host unshard: sum the two head-shards of each batch, add the folded bias
    bv = b_qkv[2 * E:]                       # v bias, folded through w_out
    b_eff = b_out + w_out @ bv               # (768,)
    y = np.empty((B, N, E), dtype=np.float32)
    for b in range(B):
        y[b] = (
            res.results[2 * b]["yp"].astype(np.float32)
            + res.results[2 * b + 1]["yp"].astype(np.float32)
            + b_eff
        )
    return y


# revision 32
# speedup vs baseline: 1.0264x; 1.0254x over previous
"""Multi-head attention (B=4, N=2048, E=768, H=12, D=64) on 8 TRN2 NeuronCores.

Sharding: data-parallel on batch (4 batches x 2 cores each), tensor-parallel on
heads (6 heads per core).  Each core computes its heads' full NxN attention.
Partial output projections from the two cores of a batch are summed on the host.

Math simplifications (all exact):
  - softmax is shift invariant -> drop the +1.0 score bias and max-subtraction
  - K bias adds a per-query constant to every score row -> softmax invariant
  - V bias folded into the output bias on the host: b_eff = b_out + w_out @ bv
  - q scaling (1/8) folded into Wq and bq on the host

Device schedule (per core): one flat software pipeline over 192 "blocks"
(12 calls = 3 head-pairs x 4 query chunks of 512; 16 key blocks per call).
Scores are computed transposed with BOTH heads of the pair packed into one
[128,1024] PSUM tile (head A scores in cols 0:512, head B in 512:1024), so the
scalar engine runs exactly ONE [128,1024] exp per block -- its 260ns/instr
overhead is amortized and the exp stream (192 x ~1.1us = 214us) stays just
under the PE's total work.  Score tiles are double-buffered (2 tags x 2
banks), the O^T accumulators are [65,512] (1 bank each: 64 d rows + a
ones-column row-sum), and the remaining 2 PSUM banks are DEDICATED to
projection fills, so fills never insert a DVE-copy WAR into the QK/exp
critical path.  PV runs one block behind its exp, so the PE never stalls
waiting for the scalar engine.  Softmax normalization (reciprocal row-sums,
broadcast, per-head multiply) runs on the DVE/Pool engines at call
boundaries, off both critical engines.
"""

import sys

if "/opt/trn_rl_repo" not in sys.path:
    sys.path.insert(0, "/opt/trn_rl_repo")

import numpy as np

B, N, E = 4, 2048, 768
H, D = 12, 64
HPC = 6                     # heads per core
FQK = HPC * D               # 384 q (or k) features per core
NCORES = 8
SCALE = D ** -0.5
PRECISION = "bf16"

_CACHE = {}


def _build_bass():
    """Build the SPMD Bass program (same program on all 8 cores)."""
    if "nc" in _CACHE:
        return _CACHE["nc"]

    from contextlib import ExitStack

    import concourse.bass as bass
    import concourse.tile as tile
    from concourse import bacc, mybir

    f32 = mybir.dt.float32
    fmm = mybir.dt.bfloat16
    Exp = mybir.ActivationFunctionType.Exp
    Add = mybir.AluOpType.add

    nc = bacc.Bacc(
        "TRN2",
        target_bir_lowering=False,
        debug=False,
        num_devices=NCORES,
    )

    xT = nc.dram_tensor("xT", (E, N), fmm, kind="ExternalInput").ap()        # x[b].T
    wqkT = nc.dram_tensor("wqkT", (E, 2 * FQK), fmm, kind="ExternalInput").ap()
    bq = nc.dram_tensor("bq", (FQK, 1), f32, kind="ExternalInput").ap()
    wvT = nc.dram_tensor("wvT", (E, FQK), fmm, kind="ExternalInput").ap()
    woT = nc.dram_tensor("woT", (FQK, E), fmm, kind="ExternalInput").ap()
    yp = nc.dram_tensor("yp", (N, E), fmm, kind="ExternalOutput").ap()

    P = 128
    IC = 512                # query chunk (one call accumulates O^T for 512 q)
    NB = N // P             # 16 key blocks per call
    NPAIRS = HPC // 2       # 3 head pairs
    NCALLS = NPAIRS * 4     # pair-major: call = 4*p + chunk
    VB = NCALLS * NB        # 192 blocks

    with ExitStack() as ctx:
        tc = ctx.enter_context(tile.TileContext(nc))

        # ---- persistent tiles --------------------------------------------
        wpool = ctx.enter_context(tc.tile_pool(name="w", bufs=1))
        wqk_t = [wpool.tile([P, 2 * FQK], fmm, tag=f"wqk{t}", name=f"wqk{t}")
                 for t in range(6)]
        bqt = wpool.tile([P, 3], f32, tag="bq", name="bqt")
        xe_t = [wpool.tile([P, N], fmm, tag=f"xe{t}", name=f"xe{t}")
                for t in range(6)]
        wv_t = [wpool.tile([P, FQK], fmm, tag=f"wv{t}", name=f"wv{t}")
                for t in range(6)]
        wo_t = [wpool.tile([P, E], fmm, tag=f"wo{t}", name=f"wo{t}")
                for t in range(3)]
        scratch = wpool.tile([1, 2], f32, tag="scr", name="scratch")

        qk_pool = ctx.enter_context(tc.tile_pool(name="qk", bufs=1))
        # f-blocks 0..2 = q features (heads 2fb, 2fb+1), 3..5 = k features
        qkT_t = [
            qk_pool.tile([P, N], fmm, tag=f"qk{fb}", name=f"qkT{fb}")
            for fb in range(6)
        ]
        v_pool = ctx.enter_context(tc.tile_pool(name="v", bufs=1))
        # V' per key-block: [128 keys, 6*65] = per head 64 V cols + a ones col
        v_t = [
            v_pool.tile([P, HPC * 65], fmm, tag=f"v{nb}", name=f"vv{nb}")
            for nb in range(NB)
        ]
        oT_pool = ctx.enter_context(tc.tile_pool(name="oT", bufs=1))
        # pair p partitions 0:64 = head 2p, 64:128 = head 2p+1 (normalized)
        oT_t = [
            oT_pool.tile([P, N], fmm, tag=f"oT{p}", name=f"oT{p}")
            for p in range(NPAIRS)
        ]

        # PSUM: s0,s1 = double-buffered packed score tiles (2 banks each),
        # oA,oB = O^T accumulators (1 bank each), f0,f1 = fill banks.
        psum = ctx.enter_context(tc.tile_pool(name="ps", bufs=1, space="PSUM"))

        pt_pool = ctx.enter_context(tc.tile_pool(name="pt", bufs=4))
        nrm_pool = ctx.enter_context(tc.tile_pool(name="nrm", bufs=2))
        y_pool = ctx.enter_context(tc.tile_pool(name="y", bufs=3))
        part_pool = ctx.enter_context(tc.tile_pool(name="part", bufs=8))

        # ---- ones columns for the row-sum trick (rest written by v-proj) --
        for nb in range(NB):
            v3 = v_t[nb].rearrange("p (h c) -> p h c", c=65)
            nc.gpsimd.memset(v3[:, :, 64:65], 1.0)

        # ---- input DMAs: split across the two hardware DGE queues --------
        # sync queue: weights; activation queue: x^T (in token halves so the
        # first projection chains can start before the full x has landed);
        # the gpsimd SWDGE ring carries a slice of x's first half.
        for t in range(6):
            nc.sync.dma_start(wqk_t[t][:], wqkT[t * P:(t + 1) * P, :])
        for fb in range(3):
            nc.sync.dma_start(bqt[:, fb:fb + 1], bq[fb * P:(fb + 1) * P, :])
        for t in range(6):
            nc.sync.dma_start(wv_t[t][:], wvT[t * P:(t + 1) * P, :])
        for t in range(3):
            nc.scalar.dma_start(xe_t[t][:, 0:1024], xT[t * P:(t + 1) * P, 0:1024])
        for t in range(3, 6):
            nc.gpsimd.dma_start(xe_t[t][:, 0:1024], xT[t * P:(t + 1) * P, 0:1024])
        for t in range(3):
            nc.scalar.dma_start(xe_t[t][:, 1024:1536],
                                xT[t * P:(t + 1) * P, 1024:1536])
        for t in range(3, 6):
            nc.gpsimd.dma_start(xe_t[t][:, 1024:1536],
                                xT[t * P:(t + 1) * P, 1024:1536])
        for t in range(6):
            nc.sync.dma_start(xe_t[t][:, 1536:N],
                              xT[t * P:(t + 1) * P, 1536:N])
        for t in range(3):
            nc.sync.dma_start(wo_t[t][:], woT[t * P:(t + 1) * P, :])

        # ones row for the PE-broadcast of reciprocal row-sums
        ones1 = wpool.tile([1, P], fmm, tag="ones1", name="ones1")
        nc.gpsimd.memset(ones1[:], 1.0)

        # warm the Exp activation table while DMAs run
        nc.gpsimd.memset(scratch[:], 0.0)
        nc.scalar.activation(scratch[:, 0:1], scratch[:, 1:2], Exp)

        # ---- projection fill units (dedicated psum banks f0/f1) ----------
        # Each fill is split into parts of <=2 matmuls placed on CONSECUTIVE
        # blocks: the per-block PE slack over the scalar's 1114ns exp pace is
        # only ~260ns, so a whole 6-matmul chain on one block starves the
        # exp stream.  Parts of one fill share a psum bank (accumulation),
        # assigned by interval (with a one-block cooldown for the DVE
        # copy-retire) so concurrent fills never collide.

        def qk_parts(fb, c4, tag):
            """QKV projection for f-block fb, token chunk c4: 3 parts."""
            st = {}
            n0 = c4 * IC

            def mk(k):
                def u():
                    if k == 0:
                        st["ps"] = psum.tile([P, IC], f32, tag=tag,
                                             name="ps_qk")
                    ps = st["ps"]
                    for et in (2 * k, 2 * k + 1):
                        nc.tensor.matmul(
                            ps[:],
                            lhsT=wqk_t[et][:, fb * P:(fb + 1) * P],
                            rhs=xe_t[et][:, n0:n0 + IC],
                            start=(et == 0),
                            stop=(et == 5),
                        )
                    if k == 2:
                        dst = qkT_t[fb][:, n0:n0 + IC]
                        if fb < 3:
                            nc.vector.tensor_scalar_add(dst, ps[:],
                                                        bqt[:, fb:fb + 1])
                        else:
                            nc.vector.tensor_copy(dst, ps[:])
                return u
            return [mk(k) for k in range(3)]

        def v_parts(nb, pp, tag):
            """V projection for key block nb, head pair pp (one part)."""
            def u():
                ps = psum.tile([P, 2 * D], f32, tag=tag, name="ps_v")
                for et in range(6):
                    nc.tensor.matmul(
                        ps[:],
                        lhsT=xe_t[et][:, nb * P:(nb + 1) * P],
                        rhs=wv_t[et][:, pp * 2 * D:(pp + 1) * 2 * D],
                        start=(et == 0),
                        stop=(et == 5),
                    )
                v3 = v_t[nb].rearrange("p (h c) -> p h c", c=65)
                nc.vector.tensor_copy(
                    v3[:, 2 * pp:2 * pp + 2, 0:64],
                    ps.rearrange("p (h c) -> p h c", c=64),
                )
            return [u]

        yts = {}

        def op_parts(c4, nb2, half, tag):
            """Out-projection half-block for query chunk c4 (one part)."""
            n0 = c4 * IC + nb2 * P
            f0 = half * 384

            def u():
                ps = psum.tile([P, 384], f32, tag=tag, name="ps_op")
                for dt3 in range(3):
                    nc.tensor.matmul(
                        ps[:],
                        lhsT=oT_t[dt3][:, n0:n0 + P],
                        rhs=wo_t[dt3][:, f0:f0 + 384],
                        start=(dt3 == 0),
                        stop=(dt3 == 2),
                    )
                if (c4, nb2) not in yts:
                    yts[(c4, nb2)] = y_pool.tile([P, E], fmm, tag="y",
                                                 name="yt")
                yt = yts[(c4, nb2)]
                nc.vector.tensor_copy(yt[:, f0:f0 + 384], ps[:])
                eng = nc.scalar if (2 * nb2 + half) % 2 == 1 else nc.sync
                eng.dma_start(yp[n0:n0 + P, f0:f0 + 384], yt[:, f0:f0 + 384])
            return [u]

        parts = {}

        def p1_parts(nb2, half, tag):
            """Tail out-proj (chunk c3) part 1: pairs 0/1, banked to SBUF."""
            def u():
                n0 = 3 * IC + nb2 * P
                f0 = half * 384
                ps = psum.tile([P, 384], f32, tag=tag, name="ps_p1")
                for dt3 in range(2):
                    nc.tensor.matmul(
                        ps[:],
                        lhsT=oT_t[dt3][:, n0:n0 + P],
                        rhs=wo_t[dt3][:, f0:f0 + 384],
                        start=(dt3 == 0),
                        stop=(dt3 == 1),
                    )
                pt = part_pool.tile([P, 384], f32, tag="part", name="ypart")
                parts[(nb2, half)] = pt
                nc.vector.tensor_copy(pt[:], ps[:])
            return [u]

        def op_p2(nb2, half, tag):
            """Tail out-proj part 2: pair 2 matmul + add + ship."""
            def u():
                n0 = 3 * IC + nb2 * P
                f0 = half * 384
                if (3, nb2) not in yts:
                    yts[(3, nb2)] = y_pool.tile([P, E], fmm, tag="y",
                                                name="yt")
                yt = yts[(3, nb2)]
                ps = psum.tile([P, 384], f32, tag=tag, name="ps_p2")
                nc.tensor.matmul(
                    ps[:],
                    lhsT=oT_t[2][:, n0:n0 + P],
                    rhs=wo_t[2][:, f0:f0 + 384],
                    start=True,
                    stop=True,
                )
                nc.vector.tensor_add(yt[:, f0:f0 + 384], ps[:],
                                     parts[(nb2, half)][:])
                eng = nc.scalar if (2 * nb2 + half) % 2 == 1 else nc.sync
                eng.dma_start(yp[n0:n0 + P, f0:f0 + 384], yt[:, f0:f0 + 384])
            return u

        # ---- fill schedule ------------------------------------------------
        # (builder, part_costs, release, deadline): parts go on consecutive
        # blocks in [release, deadline]; cost is matmul columns (PE time).
        QK3, V1, OP1, P11 = [1024] * 3, [768], [1152], [768]
        todo = []
        for c4 in range(2, 4):                    # kT pair 0 chunks 2..3
            todo.append((lambda t, c4=c4: qk_parts(3, c4, t),
                         QK3, 0, 4 * c4 - 1))
        for j in range(6, NB):                    # V pair 0 (0-5 in pre)
            todo.append((lambda t, j=j: v_parts(j, 0, t),
                         V1, 0 if j < 8 else 6, max(0, j - 1)))
        for j in range(NB):                       # V pairs 1/2 (later calls)
            todo.append((lambda t, j=j: v_parts(j, 1, t),
                         V1, 8, 20 + 2 * j))
        for j in range(NB):
            todo.append((lambda t, j=j: v_parts(j, 2, t),
                         V1, 8, 84 + 2 * j))
        for c4 in range(1, 4):                    # qT pair 0
            todo.append((lambda t, c4=c4: qk_parts(0, c4, t),
                         QK3, 0 if c4 < 2 else 8, 16 * c4 - 2))
        for c4 in range(4):                       # pair 1 k then q
            todo.append((lambda t, c4=c4: qk_parts(4, c4, t),
                         QK3, 0 if c4 < 2 else 8, 56 if c4 == 0 else 62 + 4 * c4))
        for c4 in range(4):
            todo.append((lambda t, c4=c4: qk_parts(1, c4, t),
                         QK3, 0 if c4 < 2 else 8, 58 if c4 == 0 else 62 + 16 * c4))
        for c4 in range(4):                       # pair 2 k then q
            todo.append((lambda t, c4=c4: qk_parts(5, c4, t),
                         QK3, 0 if c4 < 2 else 8, 120 if c4 == 0 else 126 + 4 * c4))
        for c4 in range(4):
            todo.append((lambda t, c4=c4: qk_parts(2, c4, t),
                         QK3, 0 if c4 < 2 else 8, 122 if c4 == 0 else 126 + 16 * c4))
        # out-proj chunk c: needs the pair-2 chunk-c normalization (blocks
        # 16*(9+c)+2 and +3); spread across that call.
        for c4 in range(3):
            for i in range(8):
                nb2, half = i // 2, i % 2
                todo.append((lambda t, a=c4, b=nb2, h=half: op_parts(a, b, h, t),
                             OP1, 16 * (9 + c4) + 6, 16 * (9 + c4) + 7 + 2 * i))
        for i in range(8):                        # tail p1s (calls 8-9)
            nb2, half = i // 2, i % 2
            todo.append((lambda t, b=nb2, h=half: p1_parts(b, h, t),
                         P11, 134, 142 + 2 * i))

        # Interval-based placement: EDF; for each fill pick the window in
        # [release, deadline] minimizing peak block load, requiring a psum
        # bank free over [start, end+1] (cooldown covers the DVE copy).
        load = [0] * VB
        tag_busy = {"f0": [(-10, -1)], "f1": [(-10, -1)]}  # pre-phase holds
        # normalization broadcasts: f0 two blocks after each boundary, f1
        # three blocks after (split so neither waits on the DVE chain)
        for k in range(1, NCALLS):
            tag_busy["f0"].append((k * NB + 4, k * NB + 4))
            tag_busy["f1"].append((k * NB + 5, k * NB + 5))
            load[k * NB + 4] += 512
            load[k * NB + 5] += 512
        fills = {}

        def tag_free(tag, b0, b1, soft=False):
            # hard: one spare block around each hold (covers the DVE
            # copy-retire); soft: abutting holds allowed (the later unit
            # then briefly WAR-waits the earlier fill's DVE copy)
            g = 0 if soft else 1
            return all(s > b1 + g or e < b0 - g for s, e in tag_busy[tag])

        # place the window-hungry multi-part chains first, singles after
        todo.sort(key=lambda t: (len(t[1]) == 1, t[3]))
        for builder, costs, rel, dl in todo:
            n = len(costs)
            best = None
            for b in range(rel, min(dl - n + 1, VB - n) + 1):
                score = max(load[b + i] + costs[i] for i in range(n))
                tag = next((t for t in ("f0", "f1")
                            if tag_free(t, b, b + n - 1)), None)
                if tag is None:
                    tag = next((t for t in ("f0", "f1")
                                if tag_free(t, b, b + n - 1, soft=True)),
                               None)
                    score += 600
                if tag is None:
                    continue
                if best is None or score <= best[0]:
                    best = (score, b, tag)
            assert best is not None, f"no window for fill dl={dl}"
            _, b, tag = best
            for i, u in enumerate(builder(tag)):
                fills.setdefault(b + i, []).append(u)
                load[b + i] += costs[i]
            tag_busy[tag].append((b, b + n - 1))

        # ---- pre-phase: minimum to start block 0 --------------------------
        for u in qk_parts(3, 0, "f0"):     # kT pair 0, keys 0:512
            u()
        for u in qk_parts(0, 0, "f1"):     # qT pair 0, queries 0:512
            u()
        for u in v_parts(0, 0, "f0"):
            u()
        for u in v_parts(1, 0, "f1"):
            u()
        for u in v_parts(2, 0, "f0"):
            u()
        for u in v_parts(3, 0, "f1"):
            u()
        for u in qk_parts(3, 1, "f0"):     # kT pair 0, keys 512:1024
            u()
        for u in v_parts(4, 0, "f1"):
            u()
        for u in v_parts(5, 0, "f1"):
            u()

        # ---- the flat pipeline --------------------------------------------
        def boundary(p, c4, oA, oB):
            """End of call (pair p, chunk c4): copy (unnormalized) O^T out
            of PSUM (both halves FIRST, so the next call's first PVs are
            not held up), then build bf16 reciprocal row-sum rows on the
            DVE.  Returns two deferred closures (run 2 and 3 blocks later,
            when the DVE chain has drained) that broadcast the reciprocals
            across partitions via a tiny PE ones-matmul into a fill bank
            and normalize oT in place."""
            q0 = c4 * IC
            last = (p == NPAIRS - 1 and c4 == 3)
            dsts, rcps = [], []
            if not last:
                for half, o_ps in ((0, oA), (1, oB)):
                    dst = oT_t[p][half * 64:(half + 1) * 64, q0:q0 + IC]
                    nc.vector.tensor_copy(dst, o_ps[0:64, :])
                    dsts.append(dst)
            for half, o_ps in ((0, oA), (1, oB)):
                rs = nrm_pool.tile([1, IC], f32, tag=f"rs{half}", name="rs")
                nc.vector.tensor_copy(rs[:], o_ps[64:65, :])
                rcp = nrm_pool.tile([1, IC], f32, tag=f"rcp{half}",
                                    name="rcp")
                nc.vector.reciprocal_approx_fast(rcp[:], rs[:])
                rcp16 = nrm_pool.tile([1, IC], fmm, tag=f"rcp16{half}",
                                      name="rcp16")
                nc.vector.tensor_copy(rcp16[:], rcp[:])
                rcps.append(rcp16)
                if last:
                    dst = oT_t[p][half * 64:(half + 1) * 64, q0:q0 + IC]
                    nc.vector.tensor_copy(dst, o_ps[0:64, :])
                    dsts.append(dst)

            def norm(half, tag):
                def n():
                    rb = psum.tile([P, IC], f32, tag=tag, name="rb")
                    nc.tensor.matmul(rb[:], lhsT=ones1[:], rhs=rcps[half][:],
                                     start=True, stop=True)
                    nc.vector.tensor_mul(
                        dsts[half], dsts[half],
                        rb[half * 64:(half + 1) * 64, :])
                return n
            return [norm(0, "f0"), norm(1, "f1")]

        cur = {}             # live per-call psum accumulators
        pending = []         # deferred normalization closures

        for vb in range(VB + 2):
            if vb < VB:
                c, jb = divmod(vb, NB)
                p, c4 = divmod(c, 4)
                q0 = c4 * IC
                j0 = jb * P
                qT = qkT_t[p]
                kT = qkT_t[3 + p]
                s = psum.tile([P, 2 * IC], f32, tag=f"s{vb % 2}", name="s")
                nc.tensor.matmul(
                    s[:, 0:IC],
                    lhsT=kT[0:64, j0:j0 + P],
                    rhs=qT[0:64, q0:q0 + IC],
                    start=True, stop=True,
                )
                nc.tensor.matmul(
                    s[:, IC:2 * IC],
                    lhsT=kT[64:128, j0:j0 + P],
                    rhs=qT[64:128, q0:q0 + IC],
                    start=True, stop=True,
                )
                pt = pt_pool.tile([P, 2 * IC], fmm, tag="pt", name="pt")
                nc.scalar.activation(pt[:], s[:], Exp)
                cur[vb] = (pt, p, jb)

            if vb >= 2:
                pt, pp, pjb = cur.pop(vb - 2)
                if pjb == 0:
                    cur["oA"] = psum.tile([65, IC], f32, tag="oA", name="oA")
                    cur["oB"] = psum.tile([65, IC], f32, tag="oB", name="oB")
                oA, oB = cur["oA"], cur["oB"]
                nc.tensor.matmul(
                    oA[:],
                    lhsT=v_t[pjb][:, (2 * pp) * 65:(2 * pp) * 65 + 65],
                    rhs=pt[:, 0:IC],
                    start=(pjb == 0),
                    stop=(pjb == NB - 1),
                )
                nc.tensor.matmul(
                    oB[:],
                    lhsT=v_t[pjb][:, (2 * pp + 1) * 65:(2 * pp + 1) * 65 + 65],
                    rhs=pt[:, IC:2 * IC],
                    start=(pjb == 0),
                    stop=(pjb == NB - 1),
                )
                if pjb == NB - 1:
                    pc = (vb - 2) // NB
                    pending.extend(boundary(pc // 4, pc % 4, oA, oB))

            if vb > VB - 1:
                while pending:
                    pending.pop(0)()
            elif pending and vb % NB >= 4:
                pending.pop(0)()

            for u in fills.get(vb, ()):
                u()

        # ---- tail: out-proj part 2 for the last query chunk ---------------
        TAIL_TAGS = ["f0", "f1", "s0", "s1", "oA", "oB"]
        for i in range(8):
            op_p2(i // 2, i % 2, TAIL_TAGS[i % 6])()

    nc.compile()
    _CACHE["nc"] = nc
    return nc


def _shard_inputs(x_q, w_qkv, b_qkv, w_out):
    """Build the 8 per-core input maps (numpy, host side)."""
    import ml_dtypes

    mm_np = ml_dtypes.bfloat16

    def cmm(a):
        return np.ascontiguousarray(a.astype(mm_np))

    in_maps = []
    for c in range(NCORES):
        b = c // 2
        h0 = (c % 2) * HPC
        qs = slice(h0 * D, h0 * D + FQK)
        ks = slice(E + h0 * D, E + h0 * D + FQK)
        vs = slice(2 * E + h0 * D, 2 * E + h0 * D + FQK)
        wq = w_qkv[qs] * SCALE                       # (384, 768)
        wk = w_qkv[ks]
        wv = w_qkv[vs]
        in_maps.append({
            "xT": cmm(x_q[b].T),                                     # (768, 2048)
            "wqkT": cmm(np.concatenate([wq, wk], axis=0).T),         # (768, 768)
            "bq": np.ascontiguousarray(
                (b_qkv[qs] * SCALE).reshape(FQK, 1)),                # (384, 1)
            "wvT": cmm(wv.T),                                        # (768, 384)
            "woT": cmm(w_out[:, h0 * D:h0 * D + FQK].T),
        })
    return in_maps


def kernel(x_q, w_qkv, b_qkv, w_out, b_out, _trace=False, _tmpdir=None):
    x_q = np.asarray(x_q, dtype=np.float32)
    w_qkv = np.asarray(w_qkv, dtype=np.float32)
    b_qkv = np.asarray(b_qkv, dtype=np.float32)
    w_out = np.asarray(w_out, dtype=np.float32)
    b_out = np.asarray(b_out, dtype=np.float32)

    from concourse.bass_utils import run_bass_kernel_spmd

    nc = _build_bass()
    in_maps = _shard_inputs(x_q, w_qkv, b_qkv, w_out)
    res = run_bass_kernel_spmd(
        nc, in_maps, core_ids=list(range(NCORES)), trace=_trace, tmpdir=_tmpdir
    )
    _CACHE["last_result"] = res

    # host unshard: sum the two head-shards of each batch, add the folded bias
    bv = b_qkv[2 * E:]                       # v bias, folded through w_out
    b_eff = b_out + w_out @ bv               # (768,)
    y = np.empty((B, N, E), dtype=np.float32)
    for b in range(B):
        y[b] = (
            res.results[2 * b]["yp"].astype(np.float32)
            + res.results[2 * b + 1]["yp"].astype(np.float32)
            + b_eff
        )
    return y
